# revision 2
# baseline (speedup 1.0000x reference)
import os
import sys

for _p in ("/opt/trn_rl_repo", "/root/.axon_site/_ro/trn_rl_repo"):
    if os.path.isdir(_p) and _p not in sys.path:
        sys.path.insert(0, _p)

import numpy as np

N_CORES = 8
T_FULL = 16384
T = T_FULL // N_CORES      # 2048 tokens per core
D = 7168
E = 256
KT = D // 128              # 56 contraction tiles
TT = T // 128              # 16 token tiles per core
CH = 4                     # token tiles per chunk
NCHUNK = TT // CH          # 4 chunks per core
CW = CH * 128              # 512 tokens per chunk

N_GROUPS = 8
GROUP_SIZE = E // N_GROUPS  # 32
TOPK_GROUPS = 4
TOPK = 8
ROUTE_SCALE = 2.5
NEG_BIG = 1.0e30

_NC = None
TRACE = False
LAST_RESULT = None


def _build_nc():
    import concourse.bass as bass
    import concourse.tile as tile
    from concourse import bacc, mybir

    nc = bacc.Bacc(None, target_bir_lowering=False)
    f32 = mybir.dt.float32
    f16 = mybir.dt.float16
    u32 = mybir.dt.uint32
    Alu = mybir.AluOpType

    xT = nc.dram_tensor("xT", [D, T], f16, kind="ExternalInput")
    wT = nc.dram_tensor("wT", [D, E], f16, kind="ExternalInput")
    bb = nc.dram_tensor("bb", [128, E], f32, kind="ExternalInput")
    v8o = nc.dram_tensor("v8o", [T, TOPK], f32, kind="ExternalOutput")
    i8o = nc.dram_tensor("i8o", [T, TOPK], u32, kind="ExternalOutput")

    with tile.TileContext(nc) as tc:
        with (
            tc.tile_pool(name="w", bufs=1) as wpool,
            tc.tile_pool(name="c", bufs=1) as cpool,
            tc.tile_pool(name="x", bufs=4) as xpool,
            tc.tile_pool(name="s", bufs=6) as spool,
            tc.tile_pool(name="gm", bufs=6) as gmpool,
            tc.tile_pool(name="sm", bufs=18) as smpool,
            tc.tile_pool(name="o", bufs=8) as opool,
            tc.tile_pool(name="ps", bufs=2 * CH, space=bass.MemorySpace.PSUM) as pspool,
        ):
            # bias replicated across partitions: [128, E] f32
            bt = cpool.tile([128, E], f32)
            nc.sync.dma_start(bt[:], bb[:, :])

            # resident gate weight: [128, KT, E] fp16 (w_all[p, k, e] = wT[k*128+p, e])
            wsb = wpool.tile([128, KT, E], f16)
            for k in range(KT):
                nc.sync.dma_start(wsb[:, k, :], wT[k * 128:(k + 1) * 128, :])

            for c in range(NCHUNK):
                ps = []
                for t in range(CH):
                    p = pspool.tile([128, E], f32)
                    ps.append(p)
                for k in range(KT):
                    xs = xpool.tile([128, CW], f16)
                    nc.sync.dma_start(
                        xs[:], xT[k * 128:(k + 1) * 128, c * CW:(c + 1) * CW]
                    )
                    for t in range(CH):
                        nc.tensor.matmul(
                            ps[t][:], xs[:, t * 128:(t + 1) * 128], wsb[:, k, :],
                            start=(k == 0), stop=(k == KT - 1),
                        )

                for t in range(CH):
                    tt = c * CH + t
                    # s = sigmoid(scores) + bias
                    s = spool.tile([128, E], f32)
                    nc.scalar.activation(
                        s[:], ps[t][:], mybir.ActivationFunctionType.Sigmoid
                    )
                    nc.vector.tensor_add(s[:], s[:], bt[:])

                    # top-8 per group (only first two used) -> [128, G, 8]
                    gm = gmpool.tile([128, N_GROUPS, 8], f32)
                    for g in range(N_GROUPS):
                        nc.vector.max(
                            out=gm[:, g, :],
                            in_=s[:, g * GROUP_SIZE:(g + 1) * GROUP_SIZE],
                        )
                    # group score = top1 + top2
                    gs = smpool.tile([128, N_GROUPS], f32)
                    nc.vector.tensor_add(gs[:], gm[:, :, 0], gm[:, :, 1])
                    # sorted group scores; threshold = 4th largest
                    g8 = smpool.tile([128, 8], f32)
                    nc.vector.max(out=g8[:], in_=gs[:])
                    # additive penalty per group: 0 if selected else -BIG
                    pen = smpool.tile([128, N_GROUPS], f32)
                    nc.vector.tensor_scalar(
                        pen[:], gs[:], g8[:, TOPK_GROUPS - 1:TOPK_GROUPS], None,
                        op0=Alu.is_ge,
                    )
                    nc.vector.tensor_scalar(
                        pen[:], pen[:], 1.0, NEG_BIG,
                        op0=Alu.subtract, op1=Alu.mult,
                    )
                    # mask out non-selected groups
                    for g in range(N_GROUPS):
                        nc.vector.tensor_scalar(
                            s[:, g * GROUP_SIZE:(g + 1) * GROUP_SIZE],
                            s[:, g * GROUP_SIZE:(g + 1) * GROUP_SIZE],
                            pen[:, g:g + 1], None,
                            op0=Alu.add,
                        )
                    # global top-8 values + indices
                    v8 = opool.tile([128, TOPK], f32)
                    nc.vector.max(out=v8[:], in_=s[:])
                    i8 = opool.tile([128, TOPK], u32)
                    nc.vector.max_index(out=i8[:], in_max=v8[:], in_values=s[:])
                    nc.sync.dma_start(v8o[tt * 128:(tt + 1) * 128, :], v8[:])
                    nc.sync.dma_start(i8o[tt * 128:(tt + 1) * 128, :], i8[:])

    nc.compile()
    return nc


def _get_nc():
    global _NC
    if _NC is None:
        _NC = _build_nc()
    return _NC


def kernel(x: np.ndarray, weight: np.ndarray, bias: np.ndarray):
    global LAST_RESULT
    from concourse import bass_utils

    nc = _get_nc()
    bias = bias.astype(np.float32)
    x16 = x.astype(np.float16)
    wT16 = np.ascontiguousarray(weight.astype(np.float16).T)  # [D, E]
    bb = np.ascontiguousarray(
        np.broadcast_to(bias[None, :], (128, E)), dtype=np.float32
    )
    in_maps = []
    for i in range(N_CORES):
        in_maps.append({
            "xT": np.ascontiguousarray(x16[i * T:(i + 1) * T].T),
            "wT": wT16,
            "bb": bb,
        })
    res = bass_utils.run_bass_kernel_spmd(
        nc, in_maps, core_ids=list(range(N_CORES)), trace=TRACE
    )
    LAST_RESULT = res
    v8 = np.concatenate([r["v8o"] for r in res.results], axis=0)  # [T_FULL, 8] f32
    i8 = np.concatenate([r["i8o"] for r in res.results], axis=0)  # [T_FULL, 8] u32
    idx = i8.astype(np.int32)
    sig8 = v8 - bias[idx]
    w8 = sig8 / sig8.sum(axis=-1, keepdims=True)
    w8 = (w8 * ROUTE_SCALE).astype(np.float32)
    return w8, idx


# revision 7
# speedup vs baseline: 1.4031x; 1.4031x over previous
import os
import sys

for _p in ("/opt/trn_rl_repo", "/root/.axon_site/_ro/trn_rl_repo"):
    if os.path.isdir(_p) and _p not in sys.path:
        sys.path.insert(0, _p)

import numpy as np

N_CORES = 8
T_FULL = 16384
T = T_FULL // N_CORES      # 2048 tokens per core
D = 7168
E = 256
KT = D // 128              # 56 contraction tiles
TT = T // 128              # 16 token tiles per core
CW = 256                   # tokens per chunk
CH = CW // 128             # token tiles per chunk (2)
NCHUNK = T // CW           # 8 chunks per core

N_GROUPS = 8
GROUP_SIZE = E // N_GROUPS  # 32
TOPK_GROUPS = 4
TOPK = 8
ROUTE_SCALE = 2.5
NEG_BIG = 1.0e30

# f32r_up16: DMA x/w as fp16, upcast on-chip, fp32r matmuls (1 cyc/row)
# f16:      DMA x/w as fp16, fp16 matmuls directly (2 cyc/row)
MODE = os.environ.get("KMODE", "f32r_up16")

_NC = None
TRACE = False
LAST_RESULT = None


def _build_nc():
    import concourse.bass as bass
    import concourse.tile as tile
    from concourse import bacc, mybir

    nc = bacc.Bacc(None, target_bir_lowering=False)
    f32 = mybir.dt.float32
    f32r = mybir.dt.float32r
    f16 = mybir.dt.float16
    u32 = mybir.dt.uint32
    Alu = mybir.AluOpType
    upcast = MODE == "f32r_up16"

    # x packed per chunk: row (c*128+p), col (k*CW+t) = x[c*CW+t, k*128+p]
    xP = nc.dram_tensor("xP", [NCHUNK * 128, KT * CW], f16, kind="ExternalInput")
    # w packed: row p, col (k*E+e) = w[e, k*128+p]
    wP = nc.dram_tensor("wP", [128, KT * E], f16, kind="ExternalInput")
    bb = nc.dram_tensor("bb", [128, E], f32, kind="ExternalInput")
    # outputs partition-major: [p, tt*8+j] for token tt*128+p
    v8d = nc.dram_tensor("v8d", [128, TT * TOPK], f32, kind="ExternalOutput")
    i8d = nc.dram_tensor("i8d", [128, TT * TOPK], u32, kind="ExternalOutput")

    with tile.TileContext(nc) as tc:
        with (
            tc.tile_pool(name="w", bufs=1) as wpool,
            tc.tile_pool(name="c", bufs=1) as cpool,
            tc.tile_pool(name="x", bufs=2) as xpool,
            tc.tile_pool(name="xf", bufs=8) as xfpool,
            tc.tile_pool(name="s", bufs=6) as spool,
            tc.tile_pool(name="gm", bufs=6) as gmpool,
            tc.tile_pool(name="sm", bufs=18) as smpool,
            tc.tile_pool(name="o", bufs=1) as opool,
            tc.tile_pool(name="ps", bufs=2 * CH, space=bass.MemorySpace.PSUM) as pspool,
        ):
            # bias replicated across partitions
            bt = cpool.tile([128, E], f32)
            nc.sync.dma_start(bt[:], bb[:, :])

            # resident gate weight
            w16 = wpool.tile([128, KT * E], f16)
            nc.sync.dma_start(w16[:], wP[:, :])
            if upcast:
                wsb = wpool.tile([128, KT * E], f32r)
                for k in range(KT):
                    eng = (nc.scalar, nc.gpsimd, nc.vector)[k % 3]
                    if eng is nc.scalar:
                        eng.copy(wsb[:, k * E:(k + 1) * E], w16[:, k * E:(k + 1) * E])
                    else:
                        eng.tensor_copy(
                            wsb[:, k * E:(k + 1) * E], w16[:, k * E:(k + 1) * E]
                        )
                mm_dt = f32r
            else:
                wsb = w16
                mm_dt = f16

            # output staging (accumulated in SBUF, one DMA at the end)
            v8sb = opool.tile([128, TT * TOPK], f32)
            i8sb = opool.tile([128, TT * TOPK], u32)

            for c in range(NCHUNK):
                xc = xpool.tile([128, KT * CW], f16)
                nc.sync.dma_start(xc[:], xP[c * 128:(c + 1) * 128, :])

                ps = []
                for t in range(CH):
                    p = pspool.tile([128, E], f32)
                    ps.append(p)

                for k in range(KT):
                    if upcast:
                        xf = xfpool.tile([128, CW], f32r)
                        eng = (nc.scalar, nc.gpsimd, nc.vector)[k % 3]
                        if eng is nc.scalar:
                            eng.copy(xf[:], xc[:, k * CW:(k + 1) * CW])
                        else:
                            eng.tensor_copy(xf[:], xc[:, k * CW:(k + 1) * CW])
                        xsrc = xf
                        xoff = 0
                    else:
                        xsrc = xc
                        xoff = k * CW
                    for t in range(CH):
                        nc.tensor.matmul(
                            ps[t][:],
                            xsrc[:, xoff + t * 128:xoff + (t + 1) * 128],
                            wsb[:, k * E:(k + 1) * E],
                            start=(k == 0), stop=(k == KT - 1),
                        )

                for t in range(CH):
                    tt = c * CH + t
                    # s = sigmoid(scores) + bias
                    s = spool.tile([128, E], f32)
                    nc.scalar.activation(
                        s[:], ps[t][:], mybir.ActivationFunctionType.Sigmoid
                    )
                    nc.vector.tensor_add(s[:], s[:], bt[:])

                    # top-8 per group (only first two used)
                    gm = gmpool.tile([128, N_GROUPS, 8], f32)
                    for g in range(N_GROUPS):
                        nc.vector.max(
                            out=gm[:, g, :],
                            in_=s[:, g * GROUP_SIZE:(g + 1) * GROUP_SIZE],
                        )
                    # group score = top1 + top2; threshold = 4th largest
                    gs = smpool.tile([128, N_GROUPS], f32)
                    nc.vector.tensor_add(gs[:], gm[:, :, 0], gm[:, :, 1])
                    g8 = smpool.tile([128, 8], f32)
                    nc.vector.max(out=g8[:], in_=gs[:])
                    # additive penalty per group: 0 if selected else -BIG
                    pen = smpool.tile([128, N_GROUPS], f32)
                    nc.vector.tensor_scalar(
                        pen[:], gs[:], g8[:, TOPK_GROUPS - 1:TOPK_GROUPS], None,
                        op0=Alu.is_ge,
                    )
                    nc.vector.tensor_scalar(
                        pen[:], pen[:], 1.0, NEG_BIG,
                        op0=Alu.subtract, op1=Alu.mult,
                    )
                    # mask non-selected groups: one op via broadcast view
                    s3 = s[:].rearrange("p (g e) -> p g e", g=N_GROUPS)
                    nc.vector.tensor_add(
                        s3, s3,
                        pen[:].unsqueeze(2).to_broadcast([128, N_GROUPS, GROUP_SIZE]),
                    )
                    # global top-8 values + indices
                    v8 = v8sb[:, tt * TOPK:(tt + 1) * TOPK]
                    nc.vector.max(out=v8, in_=s[:])
                    i8 = i8sb[:, tt * TOPK:(tt + 1) * TOPK]
                    nc.vector.max_index(out=i8, in_max=v8, in_values=s[:])

            nc.sync.dma_start(v8d[:, :], v8sb[:])
            nc.sync.dma_start(i8d[:, :], i8sb[:])

    nc.compile()
    return nc


def _get_nc():
    global _NC
    if _NC is None:
        _NC = _build_nc()
    return _NC


def _pack_x(xi16: np.ndarray) -> np.ndarray:
    # [T, D] -> [NCHUNK*128, KT*CW] with row (c*128+p), col (k*CW+t)
    return np.ascontiguousarray(
        xi16.reshape(NCHUNK, CW, KT, 128).transpose(0, 3, 2, 1)
    ).reshape(NCHUNK * 128, KT * CW)


def kernel(x: np.ndarray, weight: np.ndarray, bias: np.ndarray):
    global LAST_RESULT
    from concourse import bass_utils

    nc = _get_nc()
    bias = bias.astype(np.float32)
    x16 = x.astype(np.float16)
    wP = np.ascontiguousarray(
        weight.astype(np.float16).T.reshape(KT, 128, E).transpose(1, 0, 2)
    ).reshape(128, KT * E)
    bb = np.ascontiguousarray(
        np.broadcast_to(bias[None, :], (128, E)), dtype=np.float32
    )
    in_maps = []
    for i in range(N_CORES):
        in_maps.append({
            "xP": _pack_x(x16[i * T:(i + 1) * T]),
            "wP": wP,
            "bb": bb,
        })
    res = bass_utils.run_bass_kernel_spmd(
        nc, in_maps, core_ids=list(range(N_CORES)), trace=TRACE
    )
    LAST_RESULT = res
    # unpack outputs: [128, TT*8] -> [T, 8] with token tt*128+p
    v8 = np.concatenate(
        [r["v8d"].reshape(128, TT, TOPK).transpose(1, 0, 2).reshape(T, TOPK)
         for r in res.results], axis=0)
    i8 = np.concatenate(
        [r["i8d"].reshape(128, TT, TOPK).transpose(1, 0, 2).reshape(T, TOPK)
         for r in res.results], axis=0)
    idx = i8.astype(np.int32)
    sig8 = v8 - bias[idx]
    w8 = sig8 / sig8.sum(axis=-1, keepdims=True)
    w8 = (w8 * ROUTE_SCALE).astype(np.float32)
    return w8, idx


# revision 8
# speedup vs baseline: 2.2960x; 1.6364x over previous
import os
import sys

for _p in ("/opt/trn_rl_repo", "/root/.axon_site/_ro/trn_rl_repo"):
    if os.path.isdir(_p) and _p not in sys.path:
        sys.path.insert(0, _p)

import numpy as np

N_CORES = 8
T_FULL = 16384
T = T_FULL // N_CORES      # 2048 tokens per core
D = 7168
E = 256
KT = D // 128              # 56 contraction tiles
TT = T // 128              # 16 token tiles per core
CW = 256                   # tokens per chunk
CH = CW // 128             # token tiles per chunk (2)
NCHUNK = T // CW           # 8 chunks per core
WSPLIT = 4                 # weight DMA split (faster ramp)

N_GROUPS = 8
GROUP_SIZE = E // N_GROUPS  # 32
TOPK_GROUPS = 4
TOPK = 8
ROUTE_SCALE = 2.5
NEG_BIG = 1.0e30

_NC = None
TRACE = False
LAST_RESULT = None


def _build_nc():
    import concourse.bass as bass
    import concourse.tile as tile
    from concourse import bacc, mybir

    nc = bacc.Bacc(None, target_bir_lowering=False)
    f32 = mybir.dt.float32
    f16 = mybir.dt.float16
    u32 = mybir.dt.uint32
    Alu = mybir.AluOpType

    # x packed per chunk: row (c*128+p), col (k*CW+t) = x[c*CW+t, k*128+p]
    xP = nc.dram_tensor("xP", [NCHUNK * 128, KT * CW], f16, kind="ExternalInput")
    # w packed: row p, col (k*E+e) = w[e, k*128+p]
    wP = nc.dram_tensor("wP", [128, KT * E], f16, kind="ExternalInput")
    bb = nc.dram_tensor("bb", [128, E], f32, kind="ExternalInput")
    # outputs partition-major: [p, tt*8+j] for token tt*128+p
    v8d = nc.dram_tensor("v8d", [128, TT * TOPK], f32, kind="ExternalOutput")
    i8d = nc.dram_tensor("i8d", [128, TT * TOPK], u32, kind="ExternalOutput")

    with tile.TileContext(nc) as tc:
        with (
            tc.tile_pool(name="w", bufs=1) as wpool,
            tc.tile_pool(name="c", bufs=1) as cpool,
            tc.tile_pool(name="x", bufs=2) as xpool,
            tc.tile_pool(name="s", bufs=6) as spool,
            tc.tile_pool(name="gm", bufs=6) as gmpool,
            tc.tile_pool(name="sm", bufs=18) as smpool,
            tc.tile_pool(name="o", bufs=1) as opool,
            tc.tile_pool(name="ps", bufs=2 * CH, space=bass.MemorySpace.PSUM) as pspool,
        ):
            # bias replicated across partitions
            bt = cpool.tile([128, E], f32)
            nc.sync.dma_start(bt[:], bb[:, :])

            # resident gate weight (split DMA so first matmuls start sooner)
            wsb = wpool.tile([128, KT * E], f16)
            wstep = KT * E // WSPLIT
            for j in range(WSPLIT):
                nc.sync.dma_start(
                    wsb[:, j * wstep:(j + 1) * wstep],
                    wP[:, j * wstep:(j + 1) * wstep],
                )

            # output staging (accumulated in SBUF, one DMA at the end)
            v8sb = opool.tile([128, TT * TOPK], f32)
            i8sb = opool.tile([128, TT * TOPK], u32)

            for c in range(NCHUNK):
                xc = xpool.tile([128, KT * CW], f16)
                nc.sync.dma_start(xc[:], xP[c * 128:(c + 1) * 128, :])

                ps = []
                for t in range(CH):
                    p = pspool.tile([128, E], f32)
                    ps.append(p)

                # k-inner: 56 consecutive matmuls accumulate into ONE psum
                # bank (no psum-queue cycling between matmuls)
                for t in range(CH):
                    for k in range(KT):
                        nc.tensor.matmul(
                            ps[t][:],
                            xc[:, k * CW + t * 128:k * CW + (t + 1) * 128],
                            wsb[:, k * E:(k + 1) * E],
                            start=(k == 0), stop=(k == KT - 1),
                        )

                for t in range(CH):
                    tt = c * CH + t
                    # s = sigmoid(scores) + bias
                    s = spool.tile([128, E], f32)
                    nc.scalar.activation(
                        s[:], ps[t][:], mybir.ActivationFunctionType.Sigmoid
                    )
                    nc.vector.tensor_add(s[:], s[:], bt[:])

                    # top-8 per group (only first two used)
                    gm = gmpool.tile([128, N_GROUPS, 8], f32)
                    for g in range(N_GROUPS):
                        nc.vector.max(
                            out=gm[:, g, :],
                            in_=s[:, g * GROUP_SIZE:(g + 1) * GROUP_SIZE],
                        )
                    # group score = top1 + top2; threshold = 4th largest
                    gs = smpool.tile([128, N_GROUPS], f32)
                    nc.vector.tensor_add(gs[:], gm[:, :, 0], gm[:, :, 1])
                    g8 = smpool.tile([128, 8], f32)
                    nc.vector.max(out=g8[:], in_=gs[:])
                    # additive penalty per group: 0 if selected else -BIG
                    pen = smpool.tile([128, N_GROUPS], f32)
                    nc.vector.tensor_scalar(
                        pen[:], gs[:], g8[:, TOPK_GROUPS - 1:TOPK_GROUPS], None,
                        op0=Alu.is_ge,
                    )
                    nc.vector.tensor_scalar(
                        pen[:], pen[:], 1.0, NEG_BIG,
                        op0=Alu.subtract, op1=Alu.mult,
                    )
                    # mask non-selected groups: one op via broadcast view
                    s3 = s[:].rearrange("p (g e) -> p g e", g=N_GROUPS)
                    nc.vector.tensor_add(
                        s3, s3,
                        pen[:].unsqueeze(2).to_broadcast([128, N_GROUPS, GROUP_SIZE]),
                    )
                    # global top-8 values + indices
                    v8 = v8sb[:, tt * TOPK:(tt + 1) * TOPK]
                    nc.vector.max(out=v8, in_=s[:])
                    i8 = i8sb[:, tt * TOPK:(tt + 1) * TOPK]
                    nc.vector.max_index(out=i8, in_max=v8, in_values=s[:])

            nc.sync.dma_start(v8d[:, :], v8sb[:])
            nc.sync.dma_start(i8d[:, :], i8sb[:])

    nc.compile()
    return nc


def _get_nc():
    global _NC
    if _NC is None:
        _NC = _build_nc()
    return _NC


def _pack_x(xi16: np.ndarray) -> np.ndarray:
    # [T, D] -> [NCHUNK*128, KT*CW] with row (c*128+p), col (k*CW+t)
    return np.ascontiguousarray(
        xi16.reshape(NCHUNK, CW, KT, 128).transpose(0, 3, 2, 1)
    ).reshape(NCHUNK * 128, KT * CW)


def kernel(x: np.ndarray, weight: np.ndarray, bias: np.ndarray):
    global LAST_RESULT
    from concourse import bass_utils

    nc = _get_nc()
    bias = bias.astype(np.float32)
    x16 = x.astype(np.float16)
    wP = np.ascontiguousarray(
        weight.astype(np.float16).T.reshape(KT, 128, E).transpose(1, 0, 2)
    ).reshape(128, KT * E)
    bb = np.ascontiguousarray(
        np.broadcast_to(bias[None, :], (128, E)), dtype=np.float32
    )
    in_maps = []
    for i in range(N_CORES):
        in_maps.append({
            "xP": _pack_x(x16[i * T:(i + 1) * T]),
            "wP": wP,
            "bb": bb,
        })
    res = bass_utils.run_bass_kernel_spmd(
        nc, in_maps, core_ids=list(range(N_CORES)), trace=TRACE
    )
    LAST_RESULT = res
    # unpack outputs: [128, TT*8] -> [T, 8] with token tt*128+p
    v8 = np.concatenate(
        [r["v8d"].reshape(128, TT, TOPK).transpose(1, 0, 2).reshape(T, TOPK)
         for r in res.results], axis=0)
    i8 = np.concatenate(
        [r["i8d"].reshape(128, TT, TOPK).transpose(1, 0, 2).reshape(T, TOPK)
         for r in res.results], axis=0)
    idx = i8.astype(np.int32)
    sig8 = v8 - bias[idx]
    w8 = sig8 / sig8.sum(axis=-1, keepdims=True)
    w8 = (w8 * ROUTE_SCALE).astype(np.float32)
    return w8, idx
